# revision 23
# baseline (speedup 1.0000x reference)
"""Trainium2 Bass kernel for nn_MHA_34050500723480.

MHA forward: out = softmax((x@Wq)(x@Wk)^T / 128 + mask*-1e9) @ (x@Wv) @ W_out

Sharding: 8 cores = 2 batches x 4 head-groups (4 heads of dim 128 each).
Each core computes its batch's attention for its 4 heads plus the
row-parallel slice of out_proj; host sums the 4 partial out_proj results
per batch and adds the (v-bias @ W_out + b_out) constant.

Device-side layouts are fully "transposed" (feature dim on partitions):
host passes x^T, kernel produces q^T/k^T [d, S], v [S, d], scores^T
[keys, q] (so the key mask is a per-partition bias on the exp pass and
the PV matmul consumes exp tiles directly), and out^T [e, q] which the
host transposes back. No on-device transposes anywhere.

All matmul inputs are float32r (f32 bits, full PE rate). The softmax
division tail (reciprocal -> partition-broadcast -> multiply) is
software-pipelined one (head, q-chunk) behind the matmul chains so the
in-order PE queue never waits on the DVE reciprocal.
"""

import numpy as np

# Problem shapes (hardcoded per contract).
B = 2
S = 2048
E = 2048
D = 128          # head dim
HPC = 4          # heads per core
W = HPC * D      # 512: per-core width of q/k/v
ET = E // 128    # 16 contraction tiles for proj
SC = S // 512    # 4 s-chunks
TB = S // 128    # 16 key blocks
QC = S // 512    # 4 q-chunks
EB = E // 128    # 16 output e-blocks
CT = W // 128    # 4 contraction tiles for out proj

_CACHE = {}


def _build_nc():
    """Build (once) the single-core Bass/Tile program shared by all 8 cores."""
    from contextlib import ExitStack

    import concourse.bass as bass  # noqa: F401  (import side effects)
    import concourse.mybir as mybir
    import concourse.tile as tile
    from concourse import bacc

    dt = mybir.dt
    f32 = dt.float32
    f32r = dt.float32r
    Exp = mybir.ActivationFunctionType.Exp

    nc = bacc.Bacc("TRN2", target_bir_lowering=False, debug=False, num_devices=8)

    xc_d = nc.dram_tensor("xc", (SC, 128, ET, 512), f32r, kind="ExternalInput").ap()
    wq_d = nc.dram_tensor("wq", (HPC, 128, ET, 128), f32r, kind="ExternalInput").ap()
    wk_d = nc.dram_tensor("wk", (HPC, 128, ET, 128), f32r, kind="ExternalInput").ap()
    wv_d = nc.dram_tensor("wv", (ET, 128, W), f32r, kind="ExternalInput").ap()
    wo_d = nc.dram_tensor("wo", (EB, 128, CT, 128), f32r, kind="ExternalInput").ap()
    zt_d = nc.dram_tensor("zt", (128, TB), f32r, kind="ExternalInput").ap()
    bq_d = nc.dram_tensor("bq", (128, HPC), f32, kind="ExternalInput").ap()
    bk_d = nc.dram_tensor("bk", (128, HPC), f32, kind="ExternalInput").ap()
    out_d = nc.dram_tensor("out", (EB, 128, S), f32, kind="ExternalOutput").ap()

    with tile.TileContext(nc) as tc, ExitStack() as top:
        const = top.enter_context(tc.tile_pool(name="const", bufs=1))
        persist = top.enter_context(tc.tile_pool(name="persist", bufs=1))

        zt_t = const.tile([128, TB], f32r)   # 1-mask per key: zeros masked keys
        nc.sync.dma_start(zt_t[:], zt_d[:])
        bq_t = const.tile([128, HPC], f32)
        nc.sync.dma_start(bq_t[:], bq_d[:])
        bk_t = const.tile([128, HPC], f32)
        nc.sync.dma_start(bk_t[:], bk_d[:])

        qT = persist.tile([128, HPC, S], f32r)    # q^T per head: [d, s]
        kT = persist.tile([128, HPC, S], f32r)

        # ---------------- Phase A: qkv projection (single pass) ----------------
        # q/k weights fully SBUF-resident (8 MiB, loaded once on the scalar
        # queue); x^T chunks streamed once on the sync queue; wv streamed per
        # chunk (scalar). v tiles are masked (z = 1-mask zeroes masked keys)
        # and spilled to DRAM scratch; phase B re-streams them per head.
        dramp = top.enter_context(tc.tile_pool(name="dram", bufs=1, space="DRAM"))
        v_dram = dramp.tile([TB, 128, W], f32r)

        with ExitStack() as pa1:
            wqk_pool = pa1.enter_context(tc.tile_pool(name="wqk", bufs=1))
            xpool = pa1.enter_context(tc.tile_pool(name="xc", bufs=2))
            wvpool = pa1.enter_context(tc.tile_pool(name="wv", bufs=4))
            vb_pool = pa1.enter_context(tc.tile_pool(name="vb", bufs=3))
            qk_ps = pa1.enter_context(tc.tile_pool(name="qkps", bufs=4, space="PSUM"))
            v_ps = pa1.enter_context(tc.tile_pool(name="vps", bufs=4, space="PSUM"))

            wq_res = []
            wk_res = []
            for h in range(HPC):
                t = wqk_pool.tile([128, ET, 128], f32r, tag=f"wq{h}",
                                  name=f"wq_res{h}")
                nc.scalar.dma_start(t[:], wq_d[h])
                wq_res.append(t)
            for h in range(HPC):
                t = wqk_pool.tile([128, ET, 128], f32r, tag=f"wk{h}",
                                  name=f"wk_res{h}")
                nc.scalar.dma_start(t[:], wk_d[h])
                wk_res.append(t)

            xtiles = {}

            def load_chunk(sc):
                xt = xpool.tile([128, ET, 512], f32r, tag="xc", name=f"xt_{sc}")
                nc.sync.dma_start(xt[:], xc_d[sc])
                xtiles[sc] = xt

            load_chunk(0)
            for sc in range(SC):
                if sc + 1 < SC:
                    load_chunk(sc + 1)
                xt = xtiles.pop(sc)
                # q/k projection: out q^T/k^T block [d=128, s=512]
                for wres, dest, bias in ((wq_res, qT, bq_t), (wk_res, kT, bk_t)):
                    for h in range(HPC):
                        ps = qk_ps.tile([128, 512], f32, tag="qk")
                        for et in range(ET):
                            nc.tensor.matmul(
                                ps[:],
                                wres[h][:, et, :],
                                xt[:, et, :],
                                start=(et == 0),
                                stop=(et == ET - 1),
                            )
                        s0 = sc * 512
                        nc.vector.tensor_scalar_add(
                            dest[:, h, s0:s0 + 512], ps[:], bias[:, h:h + 1]
                        )
                # v projection: [s-block=128, d=512], masked, spilled to DRAM
                vps = [
                    v_ps.tile([128, W], f32, tag="v", name=f"vps_{sc}_{i}")
                    for i in range(4)
                ]
                for et in range(ET):
                    wvt = wvpool.tile([128, W], f32r, tag="wv")
                    nc.scalar.dma_start(wvt[:], wv_d[et])
                    for sb in range(4):
                        nc.tensor.matmul(
                            vps[sb][:],
                            xt[:, et, sb * 128:(sb + 1) * 128],
                            wvt[:],
                            start=(et == 0),
                            stop=(et == ET - 1),
                        )
                for sb in range(4):
                    tblk = sc * 4 + sb
                    vb = vb_pool.tile([128, W], f32r, tag="vb")
                    nc.vector.tensor_scalar_mul(
                        vb[:], vps[sb][:], zt_t[:, tblk:tblk + 1].bitcast(f32)
                    )
                    nc.scalar.dma_start(v_dram[tblk], vb[:])

        # ctx lives in SBUF (allocated after phase A pools release their space)
        persist2 = top.enter_context(tc.tile_pool(name="persist2", bufs=1))
        ctx_sb = persist2.tile([128, HPC, S], f32r)  # context^T per head [d, q]

        # ---------------- Phases B+C merged: attention + out-projection ----
        # qc-outer: once a q-chunk's four heads are finalized, that chunk's
        # out-projection chains are interleaved into the next chunk's
        # attention stream (they fill PE stall slack; no separate C phase).
        with ExitStack() as pb:
            vh_pool = pb.enter_context(tc.tile_pool(name="vh", bufs=1))
            wo_pool = pb.enter_context(tc.tile_pool(name="wo", bufs=1))
            exp_pool = pb.enter_context(tc.tile_pool(name="exp", bufs=3))
            rep_pool = pb.enter_context(tc.tile_pool(name="rep", bufs=2))
            rc_pool = pb.enter_context(tc.tile_pool(name="recip", bufs=2))
            ob_pool = pb.enter_context(tc.tile_pool(name="ob", bufs=3))
            sc_ps = pb.enter_context(tc.tile_pool(name="scps", bufs=2, space="PSUM"))
            ctx_ps = pb.enter_context(tc.tile_pool(name="ctxps", bufs=2, space="PSUM"))
            den_ps = pb.enter_context(tc.tile_pool(name="denps", bufs=2, space="PSUM"))
            o_ps = pb.enter_context(tc.tile_pool(name="ops", bufs=2, space="PSUM"))

            # v tiles for all heads (masked in phase A), loaded once: 4 MiB
            vh = {}
            for h in range(HPC):
                for tb in range(TB):
                    t = vh_pool.tile([128, 128], f32r, tag=f"vh{h}_{tb}",
                                     name=f"vh_{h}_{tb}")
                    nc.sync.dma_start(
                        t[:], v_dram[tb, :, h * 128:(h + 1) * 128]
                    )
                    vh[h, tb] = t
            # all wout blocks resident (4 MiB, sync queue, loads during B)
            wo_t = {}
            for eb in range(EB):
                t = wo_pool.tile([128, CT, 128], f32r, tag=f"wo{eb}",
                                 name=f"wo_{eb}")
                nc.sync.dma_start(t[:], wo_d[eb])
                wo_t[eb] = t

            def c_chain(eb, qc):
                q0 = qc * 512
                op = o_ps.tile([128, 512], f32, tag="o")
                for ct in range(CT):
                    nc.tensor.matmul(
                        op[:],
                        wo_t[eb][:, ct, :],
                        ctx_sb[:, ct, q0:q0 + 512],
                        start=(ct == 0),
                        stop=(ct == CT - 1),
                    )
                ob = ob_pool.tile([128, 512], f32, tag="ob")
                nc.vector.tensor_copy(ob[:], op[:])
                nc.scalar.dma_start(out_d[eb, :, q0:q0 + 512], ob[:])

            finalize_prev = None
            for qc in range(QC):
                for h in range(HPC):
                    q0 = qc * 512
                    ctxp = ctx_ps.tile([128, 512], f32, tag="ctx")
                    denp = den_ps.tile([1, 512], f32, tag="den")

                    def emit_pv_den(ex, tb, ctxp=ctxp, denp=denp, h=h):
                        nc.tensor.matmul(
                            ctxp[:],
                            vh[h, tb][:],
                            ex[:],
                            start=(tb == 0),
                            stop=(tb == TB - 1),
                        )
                        nc.tensor.matmul(
                            denp[:],
                            zt_t[:, tb:tb + 1],
                            ex[:],
                            start=(tb == 0),
                            stop=(tb == TB - 1),
                        )

                    # software pipeline: scores+exp of block tb emitted before
                    # PV/den of block tb-1 (ACT exp overlaps PE consumption)
                    ex_prev = None
                    for tb in range(TB):
                        sp = sc_ps.tile([128, 512], f32, tag="sc")
                        nc.tensor.matmul(
                            sp[:],
                            kT[:, h, tb * 128:(tb + 1) * 128],
                            qT[:, h, q0:q0 + 512],
                            start=True,
                            stop=True,
                        )
                        ex = exp_pool.tile([128, 512], f32r, tag="exp")
                        nc.scalar.activation(ex[:], sp[:], Exp, scale=1.0 / D)
                        if ex_prev is not None:
                            emit_pv_den(*ex_prev)
                        ex_prev = (ex, tb)
                    emit_pv_den(*ex_prev)

                    # division tail, one (h,qc) behind
                    if finalize_prev is not None:
                        finalize_prev()

                    def finalize(ctxp=ctxp, denp=denp, h=h, q0=q0):
                        rc = rc_pool.tile([1, 512], f32, tag="rc")
                        nc.vector.reciprocal(rc[:], denp[:])
                        rs = rep_pool.tile([128, 512], f32, tag="rep")
                        nc.gpsimd.partition_broadcast(rs[:], rc[:])
                        nc.vector.tensor_tensor(
                            ctx_sb[:, h, q0:q0 + 512], ctxp[:], rs[:],
                            mybir.AluOpType.mult,
                        )

                    finalize_prev = finalize
                    # out-projection chains of the previous q-chunk: 4 per
                    # attention chain (16 spread across this q-chunk's heads)
                    if qc > 0:
                        for eb in range(h * 4, h * 4 + 4):
                            c_chain(eb, qc - 1)
            finalize_prev()
            # tail: out-projection of the last q-chunk
            for eb in range(EB):
                c_chain(eb, QC - 1)

    nc.compile()
    return nc


def get_nc():
    if "nc" not in _CACHE:
        _CACHE["nc"] = _build_nc()
    return _CACHE["nc"]


def shard_inputs(c, x, mask, W_qkv, b_qkv):
    """Per-core input map (numpy f32, laid out so every device DMA is linear)."""
    b, g = divmod(c, 4)
    xT = np.ascontiguousarray(x[b].T)  # [E, S]
    xc = np.ascontiguousarray(
        xT.reshape(ET, 128, SC, 512).transpose(2, 1, 0, 3)
    )
    qs = W_qkv[:, g * W:(g + 1) * W]
    ks = W_qkv[:, E + g * W:E + (g + 1) * W]
    vs = W_qkv[:, 2 * E + g * W:2 * E + (g + 1) * W]
    wq = np.ascontiguousarray(qs.reshape(ET, 128, HPC, 128).transpose(2, 1, 0, 3))
    wk = np.ascontiguousarray(ks.reshape(ET, 128, HPC, 128).transpose(2, 1, 0, 3))
    wv = np.ascontiguousarray(vs.reshape(ET, 128, W))
    wo = np.ascontiguousarray(
        _CACHE["W_out"][g * W:(g + 1) * W, :]
        .reshape(CT, 128, EB, 128).transpose(2, 1, 0, 3)
    )
    zt = np.float32(1.0) - np.ascontiguousarray(mask[b].reshape(TB, 128).T)
    bq = np.ascontiguousarray(b_qkv[g * W:(g + 1) * W].reshape(HPC, 128).T)
    bk = np.ascontiguousarray(b_qkv[E + g * W:E + (g + 1) * W].reshape(HPC, 128).T)
    return dict(xc=xc, wq=wq, wk=wk, wv=wv, wo=wo, zt=zt, bq=bq, bk=bk)


def run(inputs, trace=False, trace_kwargs=None):
    """Run on 8 cores; returns (full output [B,S,E] f32, BassKernelResults)."""
    from concourse import bass_utils

    x = np.asarray(inputs["x"], dtype=np.float32)
    mask = np.asarray(inputs["mask"], dtype=np.float32)
    W_qkv = np.asarray(inputs["W_qkv"], dtype=np.float32)
    b_qkv = np.asarray(inputs["b_qkv"], dtype=np.float32)
    W_out = np.asarray(inputs["W_out"], dtype=np.float32)
    b_out = np.asarray(inputs["b_out"], dtype=np.float32)

    _CACHE["W_out"] = W_out
    nc = get_nc()
    in_maps = [shard_inputs(c, x, mask, W_qkv, b_qkv) for c in range(8)]
    res = bass_utils.run_bass_kernel_spmd(
        nc, in_maps, core_ids=list(range(8)), trace=trace,
        **(trace_kwargs or {}),
    )

    out_full = np.zeros((B, S, E), np.float32)
    for c, r in enumerate(res.results):
        b, _g = divmod(c, 4)
        o = r["out"]  # [EB, 128, S] = out^T partial
        out_full[b] += o.transpose(2, 0, 1).reshape(S, E)
    bv = b_qkv[2 * E:]
    out_full += (bv @ W_out + b_out)[None, None, :]
    return out_full, res


def kernel(**inputs) -> np.ndarray:
    return run(inputs, trace=False)[0]


# revision 24
# speedup vs baseline: 1.1747x; 1.1747x over previous
"""Trainium2 Bass kernel for nn_MHA_34050500723480.

MHA forward: out = softmax((x@Wq)(x@Wk)^T / 128 + mask*-1e9) @ (x@Wv) @ W_out

Sharding: 8 cores = 2 batches x 4 head-groups (4 heads of dim 128 each).
Each core computes its batch's attention for its 4 heads plus the
row-parallel slice of out_proj; host sums the 4 partial out_proj results
per batch and adds the (v-bias @ W_out + b_out) constant.

Device-side layouts are fully "transposed" (feature dim on partitions):
host passes x^T, kernel produces q^T/k^T [d, S], v [S, d], scores^T
[keys, q] (so the key mask is a per-partition bias on the exp pass and
the PV matmul consumes exp tiles directly), and out^T [e, q] which the
host transposes back. No on-device transposes anywhere.

All matmul inputs are float32r (f32 bits, full PE rate). The softmax
division tail (reciprocal -> partition-broadcast -> multiply) is
software-pipelined one (head, q-chunk) behind the matmul chains so the
in-order PE queue never waits on the DVE reciprocal.
"""

import os
import sys

import numpy as np

# kernel.py is self-contained: make the Bass/concourse stack importable
# regardless of the directory this module is loaded from.
for _p in ("/opt/trn_rl_repo",):
    if os.path.isdir(_p) and _p not in sys.path:
        sys.path.insert(0, _p)

# Problem shapes (hardcoded per contract).
B = 2
S = 2048
E = 2048
D = 128          # head dim
HPC = 4          # heads per core
W = HPC * D      # 512: per-core width of q/k/v
ET = E // 128    # 16 contraction tiles for proj
SC = S // 512    # 4 s-chunks
TB = S // 128    # 16 key blocks
QC = S // 512    # 4 q-chunks
EB = E // 128    # 16 output e-blocks
CT = W // 128    # 4 contraction tiles for out proj

_CACHE = {}


def _build_nc():
    """Build (once) the single-core Bass/Tile program shared by all 8 cores."""
    from contextlib import ExitStack

    import concourse.bass as bass  # noqa: F401  (import side effects)
    import concourse.mybir as mybir
    import concourse.tile as tile
    from concourse import bacc

    dt = mybir.dt
    f32 = dt.float32
    f32r = dt.float32r
    Exp = mybir.ActivationFunctionType.Exp

    nc = bacc.Bacc("TRN2", target_bir_lowering=False, debug=False, num_devices=8)

    xc_d = nc.dram_tensor("xc", (SC, 128, ET, 512), f32r, kind="ExternalInput").ap()
    wq_d = nc.dram_tensor("wq", (HPC, 128, ET, 128), f32r, kind="ExternalInput").ap()
    wk_d = nc.dram_tensor("wk", (HPC, 128, ET, 128), f32r, kind="ExternalInput").ap()
    wv_d = nc.dram_tensor("wv", (ET, 128, W), f32r, kind="ExternalInput").ap()
    wo_d = nc.dram_tensor("wo", (EB, 128, CT, 128), f32r, kind="ExternalInput").ap()
    zt_d = nc.dram_tensor("zt", (128, TB), f32r, kind="ExternalInput").ap()
    bq_d = nc.dram_tensor("bq", (128, HPC), f32, kind="ExternalInput").ap()
    bk_d = nc.dram_tensor("bk", (128, HPC), f32, kind="ExternalInput").ap()
    out_d = nc.dram_tensor("out", (EB, 128, S), f32, kind="ExternalOutput").ap()

    with tile.TileContext(nc) as tc, ExitStack() as top:
        const = top.enter_context(tc.tile_pool(name="const", bufs=1))
        persist = top.enter_context(tc.tile_pool(name="persist", bufs=1))

        zt_t = const.tile([128, TB], f32r)   # 1-mask per key: zeros masked keys
        nc.sync.dma_start(zt_t[:], zt_d[:])
        bq_t = const.tile([128, HPC], f32)
        nc.sync.dma_start(bq_t[:], bq_d[:])
        bk_t = const.tile([128, HPC], f32)
        nc.sync.dma_start(bk_t[:], bk_d[:])

        qT = persist.tile([128, HPC, S], f32r)    # q^T per head: [d, s]
        kT = persist.tile([128, HPC, S], f32r)

        # ---------------- Phase A: qkv projection (single pass) ----------------
        # q/k weights fully SBUF-resident (8 MiB, loaded once on the scalar
        # queue); x^T chunks streamed once on the sync queue; wv streamed per
        # chunk (scalar). v tiles are masked (z = 1-mask zeroes masked keys)
        # and spilled to DRAM scratch; phase B re-streams them per head.
        dramp = top.enter_context(tc.tile_pool(name="dram", bufs=1, space="DRAM"))
        v_dram = dramp.tile([TB, 128, W], f32r)

        with ExitStack() as pa1:
            wqk_pool = pa1.enter_context(tc.tile_pool(name="wqk", bufs=1))
            xpool = pa1.enter_context(tc.tile_pool(name="xc", bufs=2))
            wvpool = pa1.enter_context(tc.tile_pool(name="wv", bufs=4))
            vb_pool = pa1.enter_context(tc.tile_pool(name="vb", bufs=3))
            qk_ps = pa1.enter_context(tc.tile_pool(name="qkps", bufs=4, space="PSUM"))
            v_ps = pa1.enter_context(tc.tile_pool(name="vps", bufs=4, space="PSUM"))

            wq_res = []
            wk_res = []
            for h in range(HPC):
                t = wqk_pool.tile([128, ET, 128], f32r, tag=f"wq{h}",
                                  name=f"wq_res{h}")
                nc.scalar.dma_start(t[:], wq_d[h])
                wq_res.append(t)
            for h in range(HPC):
                t = wqk_pool.tile([128, ET, 128], f32r, tag=f"wk{h}",
                                  name=f"wk_res{h}")
                nc.scalar.dma_start(t[:], wk_d[h])
                wk_res.append(t)

            xtiles = {}

            def load_chunk(sc):
                xt = xpool.tile([128, ET, 512], f32r, tag="xc", name=f"xt_{sc}")
                nc.sync.dma_start(xt[:], xc_d[sc])
                xtiles[sc] = xt

            load_chunk(0)
            for sc in range(SC):
                if sc + 1 < SC:
                    load_chunk(sc + 1)
                xt = xtiles.pop(sc)
                # q/k projection: out q^T/k^T block [d=128, s=512]
                for wres, dest, bias in ((wq_res, qT, bq_t), (wk_res, kT, bk_t)):
                    for h in range(HPC):
                        ps = qk_ps.tile([128, 512], f32, tag="qk")
                        for et in range(ET):
                            nc.tensor.matmul(
                                ps[:],
                                wres[h][:, et, :],
                                xt[:, et, :],
                                start=(et == 0),
                                stop=(et == ET - 1),
                            )
                        s0 = sc * 512
                        nc.vector.tensor_scalar_add(
                            dest[:, h, s0:s0 + 512], ps[:], bias[:, h:h + 1]
                        )
                # v projection: [s-block=128, d=512], masked, spilled to DRAM
                vps = [
                    v_ps.tile([128, W], f32, tag="v", name=f"vps_{sc}_{i}")
                    for i in range(4)
                ]
                for et in range(ET):
                    wvt = wvpool.tile([128, W], f32r, tag="wv")
                    nc.scalar.dma_start(wvt[:], wv_d[et])
                    for sb in range(4):
                        nc.tensor.matmul(
                            vps[sb][:],
                            xt[:, et, sb * 128:(sb + 1) * 128],
                            wvt[:],
                            start=(et == 0),
                            stop=(et == ET - 1),
                        )
                for sb in range(4):
                    tblk = sc * 4 + sb
                    vb = vb_pool.tile([128, W], f32r, tag="vb")
                    nc.vector.tensor_scalar_mul(
                        vb[:], vps[sb][:], zt_t[:, tblk:tblk + 1].bitcast(f32)
                    )
                    nc.scalar.dma_start(v_dram[tblk], vb[:])

        # ctx lives in SBUF (allocated after phase A pools release their space)
        persist2 = top.enter_context(tc.tile_pool(name="persist2", bufs=1))
        ctx_sb = persist2.tile([128, HPC, S], f32r)  # context^T per head [d, q]
        # wout stream pool opened before phase B so its DMAs prefetch during B
        wo_pool = top.enter_context(tc.tile_pool(name="wo", bufs=4))

        # ---------------- Phase B: attention per head ----------------
        with ExitStack() as pb:
            exp_pool = pb.enter_context(tc.tile_pool(name="exp", bufs=4))
            rep_pool = pb.enter_context(tc.tile_pool(name="rep", bufs=2))
            rc_pool = pb.enter_context(tc.tile_pool(name="recip", bufs=2))
            sc_ps = pb.enter_context(tc.tile_pool(name="scps", bufs=2, space="PSUM"))
            ctx_ps = pb.enter_context(tc.tile_pool(name="ctxps", bufs=2, space="PSUM"))
            den_ps = pb.enter_context(tc.tile_pool(name="denps", bufs=2, space="PSUM"))

            vh_pool = pb.enter_context(tc.tile_pool(name="vh", bufs=2))
            vh_tiles = {}

            def load_vh(h):
                tiles = []
                for tb in range(TB):
                    t = vh_pool.tile([128, 128], f32r, tag=f"vh{tb}",
                                     name=f"vh_{h}_{tb}")
                    nc.sync.dma_start(t[:], v_dram[tb, :, h * 128:(h + 1) * 128])
                    tiles.append(t)
                vh_tiles[h] = tiles

            load_vh(0)
            finalize_prev = None
            for h in range(HPC):
                if h + 1 < HPC:
                    load_vh(h + 1)
                vh = vh_tiles.pop(h)
                for qc in range(QC):
                    q0 = qc * 512
                    ctxp = ctx_ps.tile([128, 512], f32, tag="ctx")
                    denp = den_ps.tile([1, 512], f32, tag="den")

                    def emit_pv_den(ex, tp, ctxp=ctxp, denp=denp, vh=vh):
                        for j in range(2):
                            tb = tp * 2 + j
                            nc.tensor.matmul(
                                ctxp[:],
                                vh[tb][:],
                                ex[:, j, :],
                                start=(tb == 0),
                                stop=(tb == TB - 1),
                            )
                            nc.tensor.matmul(
                                denp[:],
                                zt_t[:, tb:tb + 1],
                                ex[:, j, :],
                                start=(tb == 0),
                                stop=(tb == TB - 1),
                            )

                    # Inner software pipeline: scores+exp for pair tp are
                    # emitted before PV/den of pair tp-1, so the ACT exp of
                    # the next pair runs while the PE consumes the previous.
                    ex_prev = None
                    for tp in range(TB // 2):
                        # scores^T for two key-blocks [keys=128, 2, q=512]
                        sp = sc_ps.tile([128, 2, 512], f32, tag="sc")
                        for j in range(2):
                            tb = tp * 2 + j
                            nc.tensor.matmul(
                                sp[:, j, :],
                                kT[:, h, tb * 128:(tb + 1) * 128],
                                qT[:, h, q0:q0 + 512],
                                start=True,
                                stop=True,
                            )
                        # one exp pass over both blocks; mask needs no bias
                        # (masked keys are zeroed in v and in the z-column)
                        ex = exp_pool.tile([128, 2, 512], f32r, tag="exp")
                        nc.scalar.activation(ex[:], sp[:], Exp, scale=1.0 / D)
                        if ex_prev is not None:
                            emit_pv_den(*ex_prev)
                        ex_prev = (ex, tp)
                    emit_pv_den(*ex_prev)

                    # Division tail, pipelined one iteration behind.
                    if finalize_prev is not None:
                        finalize_prev()

                    def finalize(ctxp=ctxp, denp=denp, h=h, q0=q0):
                        rc = rc_pool.tile([1, 512], f32, tag="rc")
                        nc.vector.reciprocal(rc[:], denp[:])
                        rs = rep_pool.tile([128, 512], f32, tag="rep")
                        nc.gpsimd.partition_broadcast(rs[:], rc[:])
                        nc.vector.tensor_tensor(
                            ctx_sb[:, h, q0:q0 + 512], ctxp[:], rs[:],
                            mybir.AluOpType.mult,
                        )

                    finalize_prev = finalize
            finalize_prev()

        # ---------------- Phase C: out projection (row-parallel partial) ----------------
        with ExitStack() as pc:
            ob_pool = pc.enter_context(tc.tile_pool(name="ob", bufs=3))
            o_ps = pc.enter_context(tc.tile_pool(name="ops", bufs=6, space="PSUM"))

            wo_tiles = {}

            def load_wo(eb):
                wo_t = wo_pool.tile([128, CT, 128], f32r, tag="wo",
                                    name=f"wo_{eb}")
                nc.sync.dma_start(wo_t[:], wo_d[eb])
                wo_tiles[eb] = wo_t

            load_wo(0)
            for eb in range(EB):
                if eb + 1 < EB:
                    load_wo(eb + 1)
                wo_t = wo_tiles.pop(eb)
                ob = ob_pool.tile([128, QC, 512], f32, tag="ob")
                for qc in range(QC):
                    q0 = qc * 512
                    op = o_ps.tile([128, 512], f32, tag="o")
                    for ct in range(CT):
                        nc.tensor.matmul(
                            op[:],
                            wo_t[:, ct, :],
                            ctx_sb[:, ct, q0:q0 + 512],
                            start=(ct == 0),
                            stop=(ct == CT - 1),
                        )
                    nc.vector.tensor_copy(ob[:, qc, :], op[:])
                nc.scalar.dma_start(out_d[eb], ob[:])

    nc.compile()
    return nc


def get_nc():
    if "nc" not in _CACHE:
        _CACHE["nc"] = _build_nc()
    return _CACHE["nc"]


def shard_inputs(c, x, mask, W_qkv, b_qkv):
    """Per-core input map (numpy f32, laid out so every device DMA is linear)."""
    b, g = divmod(c, 4)
    xT = np.ascontiguousarray(x[b].T)  # [E, S]
    xc = np.ascontiguousarray(
        xT.reshape(ET, 128, SC, 512).transpose(2, 1, 0, 3)
    )
    qs = W_qkv[:, g * W:(g + 1) * W]
    ks = W_qkv[:, E + g * W:E + (g + 1) * W]
    vs = W_qkv[:, 2 * E + g * W:2 * E + (g + 1) * W]
    wq = np.ascontiguousarray(qs.reshape(ET, 128, HPC, 128).transpose(2, 1, 0, 3))
    wk = np.ascontiguousarray(ks.reshape(ET, 128, HPC, 128).transpose(2, 1, 0, 3))
    wv = np.ascontiguousarray(vs.reshape(ET, 128, W))
    wo = np.ascontiguousarray(
        _CACHE["W_out"][g * W:(g + 1) * W, :]
        .reshape(CT, 128, EB, 128).transpose(2, 1, 0, 3)
    )
    zt = np.float32(1.0) - np.ascontiguousarray(mask[b].reshape(TB, 128).T)
    bq = np.ascontiguousarray(b_qkv[g * W:(g + 1) * W].reshape(HPC, 128).T)
    bk = np.ascontiguousarray(b_qkv[E + g * W:E + (g + 1) * W].reshape(HPC, 128).T)
    return dict(xc=xc, wq=wq, wk=wk, wv=wv, wo=wo, zt=zt, bq=bq, bk=bk)


def run(inputs, trace=False, trace_kwargs=None):
    """Run on 8 cores; returns (full output [B,S,E] f32, BassKernelResults)."""
    from concourse import bass_utils

    x = np.asarray(inputs["x"], dtype=np.float32)
    mask = np.asarray(inputs["mask"], dtype=np.float32)
    W_qkv = np.asarray(inputs["W_qkv"], dtype=np.float32)
    b_qkv = np.asarray(inputs["b_qkv"], dtype=np.float32)
    W_out = np.asarray(inputs["W_out"], dtype=np.float32)
    b_out = np.asarray(inputs["b_out"], dtype=np.float32)

    _CACHE["W_out"] = W_out
    nc = get_nc()
    in_maps = [shard_inputs(c, x, mask, W_qkv, b_qkv) for c in range(8)]
    res = bass_utils.run_bass_kernel_spmd(
        nc, in_maps, core_ids=list(range(8)), trace=trace,
        **(trace_kwargs or {}),
    )

    out_full = np.zeros((B, S, E), np.float32)
    for c, r in enumerate(res.results):
        b, _g = divmod(c, 4)
        o = r["out"]  # [EB, 128, S] = out^T partial
        out_full[b] += o.transpose(2, 0, 1).reshape(S, E)
    bv = b_qkv[2 * E:]
    out_full += (bv @ W_out + b_out)[None, None, :]
    return out_full, res


def kernel(**inputs) -> np.ndarray:
    return run(inputs, trace=False)[0]


# revision 25
# speedup vs baseline: 1.2090x; 1.0292x over previous
"""Trainium2 Bass kernel for nn_MHA_34050500723480.

MHA forward: out = softmax((x@Wq)(x@Wk)^T / 128 + mask*-1e9) @ (x@Wv) @ W_out

Sharding: 8 cores = 2 batches x 4 head-groups (4 heads of dim 128 each).
Each core computes its batch's attention for its 4 heads plus the
row-parallel slice of out_proj; host sums the 4 partial out_proj results
per batch and adds the (v-bias @ W_out + b_out) constant.

Device-side layouts are fully "transposed" (feature dim on partitions):
host passes x^T, kernel produces q^T/k^T [d, S], v [S, d], scores^T
[keys, q] (so the key mask is a per-partition bias on the exp pass and
the PV matmul consumes exp tiles directly), and out^T [e, q] which the
host transposes back. No on-device transposes anywhere.

All matmul inputs are float32r (f32 bits, full PE rate). The softmax
division tail (reciprocal -> partition-broadcast -> multiply) is
software-pipelined one (head, q-chunk) behind the matmul chains so the
in-order PE queue never waits on the DVE reciprocal.
"""

import os
import sys

import numpy as np

# kernel.py is self-contained: make the Bass/concourse stack importable
# regardless of the directory this module is loaded from.
for _p in ("/opt/trn_rl_repo",):
    if os.path.isdir(_p) and _p not in sys.path:
        sys.path.insert(0, _p)

# Problem shapes (hardcoded per contract).
B = 2
S = 2048
E = 2048
D = 128          # head dim
HPC = 4          # heads per core
W = HPC * D      # 512: per-core width of q/k/v
ET = E // 128    # 16 contraction tiles for proj
SC = S // 512    # 4 s-chunks
TB = S // 128    # 16 key blocks
QC = S // 512    # 4 q-chunks
EB = E // 128    # 16 output e-blocks
CT = W // 128    # 4 contraction tiles for out proj

_CACHE = {}


def _build_nc():
    """Build (once) the single-core Bass/Tile program shared by all 8 cores."""
    from contextlib import ExitStack

    import concourse.bass as bass  # noqa: F401  (import side effects)
    import concourse.mybir as mybir
    import concourse.tile as tile
    from concourse import bacc

    dt = mybir.dt
    f32 = dt.float32
    f32r = dt.float32r
    Exp = mybir.ActivationFunctionType.Exp

    nc = bacc.Bacc("TRN2", target_bir_lowering=False, debug=False, num_devices=8)

    xc_d = nc.dram_tensor("xc", (SC, 128, ET, 512), f32r, kind="ExternalInput").ap()
    wq_d = nc.dram_tensor("wq", (HPC, 128, ET, 128), f32r, kind="ExternalInput").ap()
    wk_d = nc.dram_tensor("wk", (HPC, 128, ET, 128), f32r, kind="ExternalInput").ap()
    wv_d = nc.dram_tensor("wv", (ET, 128, W), f32r, kind="ExternalInput").ap()
    wo_d = nc.dram_tensor("wo", (EB, 128, CT, 128), f32r, kind="ExternalInput").ap()
    zt_d = nc.dram_tensor("zt", (128, TB), f32r, kind="ExternalInput").ap()
    bq_d = nc.dram_tensor("bq", (128, HPC), f32, kind="ExternalInput").ap()
    bk_d = nc.dram_tensor("bk", (128, HPC), f32, kind="ExternalInput").ap()
    out_d = nc.dram_tensor("out", (EB, 128, S), f32, kind="ExternalOutput").ap()

    with tile.TileContext(nc) as tc, ExitStack() as top:
        const = top.enter_context(tc.tile_pool(name="const", bufs=1))
        persist = top.enter_context(tc.tile_pool(name="persist", bufs=1))

        zt_t = const.tile([128, TB], f32r)   # 1-mask per key: zeros masked keys
        nc.sync.dma_start(zt_t[:], zt_d[:])
        bq_t = const.tile([128, HPC], f32)
        nc.sync.dma_start(bq_t[:], bq_d[:])
        bk_t = const.tile([128, HPC], f32)
        nc.sync.dma_start(bk_t[:], bk_d[:])

        qT = persist.tile([128, HPC, S], f32r)    # q^T per head: [d, s]
        kT = persist.tile([128, HPC, S], f32r)

        # ---------------- Phase A: qkv projection (single pass) ----------------
        # q/k weights fully SBUF-resident (8 MiB, loaded once on the scalar
        # queue); x^T chunks streamed once on the sync queue; wv streamed per
        # chunk (scalar). v tiles are masked (z = 1-mask zeroes masked keys)
        # and spilled to DRAM scratch; phase B re-streams them per head.
        dramp = top.enter_context(tc.tile_pool(name="dram", bufs=1, space="DRAM"))
        v_dram = dramp.tile([TB, 128, W], f32r)

        with ExitStack() as pa1:
            wqk_pool = pa1.enter_context(tc.tile_pool(name="wqk", bufs=1))
            xpool = pa1.enter_context(tc.tile_pool(name="xc", bufs=2))
            wvpool = pa1.enter_context(tc.tile_pool(name="wv", bufs=4))
            vb_pool = pa1.enter_context(tc.tile_pool(name="vb", bufs=3))
            qk_ps = pa1.enter_context(tc.tile_pool(name="qkps", bufs=4, space="PSUM"))
            v_ps = pa1.enter_context(tc.tile_pool(name="vps", bufs=4, space="PSUM"))

            wq_res = []
            wk_res = []
            for h in range(HPC):
                t = wqk_pool.tile([128, ET, 128], f32r, tag=f"wq{h}",
                                  name=f"wq_res{h}")
                nc.scalar.dma_start(t[:], wq_d[h])
                wq_res.append(t)
            for h in range(HPC):
                t = wqk_pool.tile([128, ET, 128], f32r, tag=f"wk{h}",
                                  name=f"wk_res{h}")
                nc.scalar.dma_start(t[:], wk_d[h])
                wk_res.append(t)

            xtiles = {}

            def load_chunk(sc):
                xt = xpool.tile([128, ET, 512], f32r, tag="xc", name=f"xt_{sc}")
                nc.sync.dma_start(xt[:], xc_d[sc])
                xtiles[sc] = xt

            load_chunk(0)
            for sc in range(SC):
                if sc + 1 < SC:
                    load_chunk(sc + 1)
                xt = xtiles.pop(sc)
                # q/k projection: out q^T/k^T block [d=128, s=512]
                for wres, dest, bias in ((wq_res, qT, bq_t), (wk_res, kT, bk_t)):
                    for h in range(HPC):
                        ps = qk_ps.tile([128, 512], f32, tag="qk")
                        for et in range(ET):
                            nc.tensor.matmul(
                                ps[:],
                                wres[h][:, et, :],
                                xt[:, et, :],
                                start=(et == 0),
                                stop=(et == ET - 1),
                            )
                        s0 = sc * 512
                        nc.vector.tensor_scalar_add(
                            dest[:, h, s0:s0 + 512], ps[:], bias[:, h:h + 1]
                        )
                # v projection: [s-block=128, d=512], masked, spilled to DRAM
                vps = [
                    v_ps.tile([128, W], f32, tag="v", name=f"vps_{sc}_{i}")
                    for i in range(4)
                ]
                for et in range(ET):
                    wvt = wvpool.tile([128, W], f32r, tag="wv")
                    nc.scalar.dma_start(wvt[:], wv_d[et])
                    for sb in range(4):
                        nc.tensor.matmul(
                            vps[sb][:],
                            xt[:, et, sb * 128:(sb + 1) * 128],
                            wvt[:],
                            start=(et == 0),
                            stop=(et == ET - 1),
                        )
                for sb in range(4):
                    tblk = sc * 4 + sb
                    vb = vb_pool.tile([128, W], f32r, tag="vb")
                    nc.vector.tensor_scalar_mul(
                        vb[:], vps[sb][:], zt_t[:, tblk:tblk + 1].bitcast(f32)
                    )
                    nc.sync.dma_start(v_dram[tblk], vb[:])

        # ctx lives in SBUF (allocated after phase A pools release their space)
        persist2 = top.enter_context(tc.tile_pool(name="persist2", bufs=1))
        ctx_sb = persist2.tile([128, HPC, S], f32r)  # context^T per head [d, q]
        # wout stream pool opened before phase B so its DMAs prefetch during B
        wo_pool = top.enter_context(tc.tile_pool(name="wo", bufs=6))

        # ---------------- Phase B: attention per head ----------------
        with ExitStack() as pb:
            exp_pool = pb.enter_context(tc.tile_pool(name="exp", bufs=6))
            rep_pool = pb.enter_context(tc.tile_pool(name="rep", bufs=2))
            rc_pool = pb.enter_context(tc.tile_pool(name="recip", bufs=2))
            sc_ps = pb.enter_context(tc.tile_pool(name="scps", bufs=2, space="PSUM"))
            ctx_ps = pb.enter_context(tc.tile_pool(name="ctxps", bufs=2, space="PSUM"))
            den_ps = pb.enter_context(tc.tile_pool(name="denps", bufs=2, space="PSUM"))

            vh_pool = pb.enter_context(tc.tile_pool(name="vh", bufs=3))
            vh_tiles = {}

            def load_vh(h):
                tiles = []
                for tb in range(TB):
                    t = vh_pool.tile([128, 128], f32r, tag=f"vh{tb}",
                                     name=f"vh_{h}_{tb}")
                    nc.sync.dma_start(t[:], v_dram[tb, :, h * 128:(h + 1) * 128])
                    tiles.append(t)
                vh_tiles[h] = tiles

            load_vh(0)
            finalize_prev = None
            for h in range(HPC):
                if h + 1 < HPC:
                    load_vh(h + 1)
                vh = vh_tiles.pop(h)
                for qc in range(QC):
                    q0 = qc * 512
                    ctxp = ctx_ps.tile([128, 512], f32, tag="ctx")
                    denp = den_ps.tile([1, 512], f32, tag="den")

                    def emit_pv_den(ex, tp, ctxp=ctxp, denp=denp, vh=vh):
                        for j in range(2):
                            tb = tp * 2 + j
                            nc.tensor.matmul(
                                ctxp[:],
                                vh[tb][:],
                                ex[:, j, :],
                                start=(tb == 0),
                                stop=(tb == TB - 1),
                            )
                            nc.tensor.matmul(
                                denp[:],
                                zt_t[:, tb:tb + 1],
                                ex[:, j, :],
                                start=(tb == 0),
                                stop=(tb == TB - 1),
                            )

                    # Inner software pipeline: scores+exp for pair tp are
                    # emitted before PV/den of pair tp-1, so the ACT exp of
                    # the next pair runs while the PE consumes the previous.
                    ex_prev = None
                    for tp in range(TB // 2):
                        # scores^T for two key-blocks [keys=128, 2, q=512]
                        sp = sc_ps.tile([128, 2, 512], f32, tag="sc")
                        for j in range(2):
                            tb = tp * 2 + j
                            nc.tensor.matmul(
                                sp[:, j, :],
                                kT[:, h, tb * 128:(tb + 1) * 128],
                                qT[:, h, q0:q0 + 512],
                                start=True,
                                stop=True,
                            )
                        # one exp pass over both blocks; mask needs no bias
                        # (masked keys are zeroed in v and in the z-column)
                        ex = exp_pool.tile([128, 2, 512], f32r, tag="exp")
                        nc.scalar.activation(ex[:], sp[:], Exp, scale=1.0 / D)
                        if ex_prev is not None:
                            emit_pv_den(*ex_prev)
                        ex_prev = (ex, tp)
                    emit_pv_den(*ex_prev)

                    # Division tail, pipelined one iteration behind.
                    if finalize_prev is not None:
                        finalize_prev()

                    def finalize(ctxp=ctxp, denp=denp, h=h, q0=q0):
                        rc = rc_pool.tile([1, 512], f32, tag="rc")
                        nc.vector.reciprocal(rc[:], denp[:])
                        rs = rep_pool.tile([128, 512], f32, tag="rep")
                        nc.gpsimd.partition_broadcast(rs[:], rc[:])
                        nc.vector.tensor_tensor(
                            ctx_sb[:, h, q0:q0 + 512], ctxp[:], rs[:],
                            mybir.AluOpType.mult,
                        )

                    finalize_prev = finalize
            finalize_prev()

        # ---------------- Phase C: out projection (row-parallel partial) ----------------
        with ExitStack() as pc:
            ob_pool = pc.enter_context(tc.tile_pool(name="ob", bufs=3))
            o_ps = pc.enter_context(tc.tile_pool(name="ops", bufs=6, space="PSUM"))

            wo_tiles = {}

            def load_wo(eb):
                wo_t = wo_pool.tile([128, CT, 128], f32r, tag="wo",
                                    name=f"wo_{eb}")
                nc.sync.dma_start(wo_t[:], wo_d[eb])
                wo_tiles[eb] = wo_t

            load_wo(0)
            for eb in range(EB):
                if eb + 1 < EB:
                    load_wo(eb + 1)
                wo_t = wo_tiles.pop(eb)
                ob = ob_pool.tile([128, QC, 512], f32, tag="ob")
                for qc in range(QC):
                    q0 = qc * 512
                    op = o_ps.tile([128, 512], f32, tag="o")
                    for ct in range(CT):
                        nc.tensor.matmul(
                            op[:],
                            wo_t[:, ct, :],
                            ctx_sb[:, ct, q0:q0 + 512],
                            start=(ct == 0),
                            stop=(ct == CT - 1),
                        )
                    nc.vector.tensor_copy(ob[:, qc, :], op[:])
                nc.scalar.dma_start(out_d[eb], ob[:])

    nc.compile()
    return nc


def get_nc():
    if "nc" not in _CACHE:
        _CACHE["nc"] = _build_nc()
    return _CACHE["nc"]


def shard_inputs(c, x, mask, W_qkv, b_qkv):
    """Per-core input map (numpy f32, laid out so every device DMA is linear)."""
    b, g = divmod(c, 4)
    xT = np.ascontiguousarray(x[b].T)  # [E, S]
    xc = np.ascontiguousarray(
        xT.reshape(ET, 128, SC, 512).transpose(2, 1, 0, 3)
    )
    qs = W_qkv[:, g * W:(g + 1) * W]
    ks = W_qkv[:, E + g * W:E + (g + 1) * W]
    vs = W_qkv[:, 2 * E + g * W:2 * E + (g + 1) * W]
    wq = np.ascontiguousarray(qs.reshape(ET, 128, HPC, 128).transpose(2, 1, 0, 3))
    wk = np.ascontiguousarray(ks.reshape(ET, 128, HPC, 128).transpose(2, 1, 0, 3))
    wv = np.ascontiguousarray(vs.reshape(ET, 128, W))
    wo = np.ascontiguousarray(
        _CACHE["W_out"][g * W:(g + 1) * W, :]
        .reshape(CT, 128, EB, 128).transpose(2, 1, 0, 3)
    )
    zt = np.float32(1.0) - np.ascontiguousarray(mask[b].reshape(TB, 128).T)
    bq = np.ascontiguousarray(b_qkv[g * W:(g + 1) * W].reshape(HPC, 128).T)
    bk = np.ascontiguousarray(b_qkv[E + g * W:E + (g + 1) * W].reshape(HPC, 128).T)
    return dict(xc=xc, wq=wq, wk=wk, wv=wv, wo=wo, zt=zt, bq=bq, bk=bk)


def run(inputs, trace=False, trace_kwargs=None):
    """Run on 8 cores; returns (full output [B,S,E] f32, BassKernelResults)."""
    from concourse import bass_utils

    x = np.asarray(inputs["x"], dtype=np.float32)
    mask = np.asarray(inputs["mask"], dtype=np.float32)
    W_qkv = np.asarray(inputs["W_qkv"], dtype=np.float32)
    b_qkv = np.asarray(inputs["b_qkv"], dtype=np.float32)
    W_out = np.asarray(inputs["W_out"], dtype=np.float32)
    b_out = np.asarray(inputs["b_out"], dtype=np.float32)

    _CACHE["W_out"] = W_out
    nc = get_nc()
    in_maps = [shard_inputs(c, x, mask, W_qkv, b_qkv) for c in range(8)]
    res = bass_utils.run_bass_kernel_spmd(
        nc, in_maps, core_ids=list(range(8)), trace=trace,
        **(trace_kwargs or {}),
    )

    out_full = np.zeros((B, S, E), np.float32)
    for c, r in enumerate(res.results):
        b, _g = divmod(c, 4)
        o = r["out"]  # [EB, 128, S] = out^T partial
        out_full[b] += o.transpose(2, 0, 1).reshape(S, E)
    bv = b_qkv[2 * E:]
    out_full += (bv @ W_out + b_out)[None, None, :]
    return out_full, res


def kernel(**inputs) -> np.ndarray:
    return run(inputs, trace=False)[0]
